# revision 17
# baseline (speedup 1.0000x reference)
"""CenterLoss update kernel for Trainium2, 8-core SPMD — class-sharded, collective-free.

Reference computation (N=16384 samples, C=10000 classes, D=128 dims):
    embeded_labels = labels @ center          # [N,D] gather via one-hot
    diff = embeded_labels - embeded_preds
    grad = (labels.T @ diff) / (counts + 1)   # counts = labels.T @ ones
    out  = center - 0.5 * grad

Because each row of ``labels`` is one-hot, ``labels.T @ labels == diag(counts)``,
so the whole thing collapses to a single pass over ``labels``:

    S      = labels.T @ embeded_preds         # [C,D] per-class sum of preds
    counts = column sums of labels            # [C]
    out    = beta * center + gamma * S
             beta  = 1 - 0.5*counts/(counts+1)
             gamma = 0.5/(counts+1)

Sharding: each core owns C/8 = 1250 classes and reads ALL N rows of its
1250-column slice of ``labels`` (same total HBM traffic as batch-sharding)
plus a replicated copy of ``preds`` (8.4MB).  No inter-core communication at
all — the batch-sharded variant needs a [C,D] ReduceScatter that shows up as
a ~100us serial tail with HBM nearly idle.

The kernel streams 91.6MB/core from HBM at ~356 GB/s (near the per-core
share of HBM bandwidth), so everything else must hide under the stream:

* The host packs each core's label slice as [4096, 5000] (4 consecutive
  sample-rows per SBUF partition line) so one super-tile DMA moves 2.56MB
  with 20KB contiguous per-partition segments.  ``preds`` is host-packed to
  match: partition p, column s*512+q*128+d holds preds[512s+4p+q, d], making
  the matmul stationary operand for sub-step (s, q) a plain SBUF slice.
* Labels stream through the PE once as the moving operand of a single fp32r
  pass (1 row/cycle; the 2e-2 rel-err budget makes the fp32 hi/lo
  double-pass unnecessary), accumulating S.T = preds.T @ labels in PSUM.
  The stationary preds must also be f32r: the BIR verifier rejects mixing
  32-bit and non-32-bit matmul inputs, so the cheap bf16 weight-load path
  is unavailable.
* Counts accumulate on the DVE in bf16 (exact for one-hot partial counts),
  pre-folding each super-tile's 4 row-blocks to 2500 columns so the final
  partition-reduce matmul has a short bf16 moving operand.
* The epilogue transposes per-class counts into a [128, 10] block so
  beta/gamma come from 4 batched DVE ops instead of 40 short ones, then
  does one ACT op + one DVE op + one store per 128-class tile.
"""

import numpy as np

N, C, D = 16384, 10000, 128
NCORES = 8
CS = C // NCORES   # 1250 classes per core
LR = 0.5
P = 128
R = 4              # sample-rows packed per SBUF partition line
ST = N // (P * R)  # 32 label super-tiles of [128, R*CS]


def _chunks(width, step=512):
    out = []
    c0 = 0
    while c0 < width:
        out.append((c0, min(step, width - c0)))
        c0 += step
    return out


# matmul chunks over the 1250-class shard, padded so every chunk is >=256
# wide: fp32r moving operands narrower than 256 run at 2-4 cycles/row.  The
# pad columns multiply garbage SBUF into PSUM columns [1250, 1280) that are
# never read.
MM_CHUNKS = [(0, 512), (512, 512), (1024, 256)]
CSP = 1280  # padded class width (PSUM tile / lab tile per-block stride)


def build_program(n=N, cs=CS, d=D, ncores=NCORES):
    """Build the SPMD Bass program (identical on every core)."""
    import concourse.bacc as bacc
    import concourse.mybir as mybir
    import concourse.tile as tile
    from concourse.masks import make_identity

    f32 = mybir.dt.float32
    f32r = mybir.dt.float32r
    bf16 = mybir.dt.bfloat16
    mult = mybir.AluOpType.mult
    add = mybir.AluOpType.add

    st = n // (P * R)          # label super-tiles
    gw = R * cs                # super-tile width (5000)
    hw_ = gw // 2              # folded counts width (2500)
    nt3 = (cs + P - 1) // P    # epilogue tiles over the class shard (10)
    npr = 4                    # preds split into 4 DMA tiles
    prw = (st * R * d) // npr  # 4096 cols per preds tile

    nc = bacc.Bacc(
        "TRN2",
        target_bir_lowering=False,
        debug=False,
        num_devices=ncores,
    )

    # labels are consumed by fp32r matmuls; declaring the DRAM tensor f32r
    # (same 32-bit values, trivially rounded) lets plain HWDGE DMAs feed the
    # PE at full speed -- the SWDGE cast path is descriptor-bound.
    # Host-side layouts (see kernel()):
    #   labels [4096, 5000]: row r = label rows 4r..4r+3 of this core's
    #                        1250-class column slice, concatenated
    #   preds  [128, 16384]: partition p, col s*512+q*128+d = preds[512s+4p+q, d]
    preds = nc.dram_tensor("preds", [P, st * R * d], f32r, kind="ExternalInput").ap()
    labels = nc.dram_tensor("labels", [st * P, gw], f32r, kind="ExternalInput").ap()
    center = nc.dram_tensor("center", [cs, d], f32, kind="ExternalInput").ap()
    out = nc.dram_tensor("out", [cs, d], f32, kind="ExternalOutput").ap()

    with tile.TileContext(nc) as tc:
        with tc.tile_pool(name="const", bufs=1) as const_pool:
            identity = const_pool.tile([P, P], f32, name="identity")
            make_identity(nc, identity[:])
            ones_col = const_pool.tile([P, 1], bf16, name="ones_col")
            nc.vector.memset(ones_col[:], 1.0)

            # preds, host-packed; 4 DMAs of ~2.1MB on the ACT HWDGE ring so
            # the label stream on the SP ring is never queued behind them.
            preds_sb = []
            for t in range(npr):
                pt = const_pool.tile([P, prw], f32r, name=f"preds_{t}")
                nc.scalar.dma_start(
                    out=pt[:], in_=preds[:, t * prw:(t + 1) * prw]
                )
                preds_sb.append(pt)

            # center shard, as nt3 [class, d] tiles (class on partitions)
            ctr_sb = const_pool.tile([P, nt3 * d], f32, name="ctr_sb")
            for tt in range(nt3):
                w = min(P, cs - tt * P)
                nc.scalar.dma_start(
                    out=ctr_sb[0:w, tt * d:tt * d + d],
                    in_=center[tt * P:tt * P + w, :],
                )

            # folded per-partition partial counts (bf16 is exact: integers
            # well under 256); col h*1250+j accumulates classes j from two of
            # the four row-blocks
            counts_sb = const_pool.tile([P, hw_], bf16, name="counts_sb")
            st_sb = const_pool.tile([d, cs], f32, name="st_sb")
            cnt_row = const_pool.tile([1, cs], f32, name="cnt_row")

            # ---------------- phase 1: stream labels ----------------
            with (
                tc.tile_pool(name="lab", bufs=4) as lab_pool,
                tc.tile_pool(name="fold", bufs=2) as fold_pool,
                tc.tile_pool(name="psum1", bufs=1, space="PSUM") as psum1,
            ):
                st_psum = psum1.tile([d, CSP], f32, name="st_psum", space="PSUM")
                for s in range(st):
                    # tile is gw+pad wide; the DMA fills [0, gw) and the
                    # padded matmul chunks read up to 30 garbage columns
                    lab_t = lab_pool.tile(
                        [P, gw + (CSP - cs)], f32r, name=f"lab_{s}", tag="lab"
                    )
                    nc.sync.dma_start(
                        out=lab_t[:, 0:gw], in_=labels[s * P:(s + 1) * P, :]
                    )
                    for q in range(R):
                        col = s * (R * d) + q * d
                        pt = preds_sb[col // prw]
                        off = col % prw
                        for c0, w in MM_CHUNKS:
                            nc.tensor.matmul(
                                out=st_psum[:, c0:c0 + w],
                                lhsT=pt[:, off:off + d],
                                rhs=lab_t[:, q * cs + c0:q * cs + c0 + w],
                                start=(s == 0 and q == 0),
                                stop=(s == st - 1 and q == R - 1),
                            )
                    # counts: fold row-blocks {0,1}+{2,3} (fp32 reads, bf16
                    # write), then accumulate bf16+bf16 at the DVE 2x rate
                    if s == 0:
                        nc.vector.tensor_tensor(
                            out=counts_sb[:],
                            in0=lab_t[:, 0:hw_].bitcast(f32),
                            in1=lab_t[:, hw_:gw].bitcast(f32),
                            op=add,
                        )
                    else:
                        tmp = fold_pool.tile([P, hw_], bf16, name=f"tmp_{s}",
                                             tag="tmp")
                        nc.vector.tensor_tensor(
                            out=tmp[:],
                            in0=lab_t[:, 0:hw_].bitcast(f32),
                            in1=lab_t[:, hw_:gw].bitcast(f32),
                            op=add,
                        )
                        nc.vector.tensor_add(
                            out=counts_sb[:], in0=counts_sb[:], in1=tmp[:]
                        )

                # counts: reduce 128 partitions with a ones matmul,
                # PSUM-accumulating the two folded halves
                cnt_psum = psum1.tile([1, cs], f32, name="cnt_psum", space="PSUM")
                for h in range(2):
                    for c0, w in _chunks(cs):
                        nc.tensor.matmul(
                            out=cnt_psum[0:1, c0:c0 + w],
                            lhsT=ones_col[:],
                            rhs=counts_sb[:, h * cs + c0:h * cs + c0 + w],
                            start=(h == 0),
                            stop=(h == 1),
                        )
                nc.scalar.copy(out=st_sb[:], in_=st_psum[:, 0:cs])
                nc.scalar.copy(out=cnt_row[:], in_=cnt_psum[:])

            # ---------------- phase 2: per-class update ----------------
            with (
                tc.tile_pool(name="p3", bufs=2) as p3,
                tc.tile_pool(name="p3c", bufs=1) as p3c,
                tc.tile_pool(name="psum3", bufs=4, space="PSUM") as psum3,
                tc.tile_pool(name="psumc", bufs=1, space="PSUM") as psumc,
            ):
                # gather per-class counts into [128, nt3] so beta/gamma are
                # 4 batched DVE ops instead of 4*nt3 short ones
                cntc = psumc.tile([P, nt3], f32, name="cntc", space="PSUM")
                for tt in range(nt3):
                    w = min(P, cs - tt * P)
                    nc.tensor.transpose(
                        out=cntc[0:w, tt:tt + 1],
                        in_=cnt_row[0:1, tt * P:tt * P + w],
                        identity=identity[0:1, 0:1],
                    )
                rec = p3c.tile([P, nt3], f32, name="rec")
                nc.vector.tensor_scalar_add(
                    out=rec[:], in0=cntc[:], scalar1=1.0
                )
                nc.vector.reciprocal(out=rec[:], in_=rec[:])
                gam = p3c.tile([P, nt3], f32, name="gam")
                nc.vector.tensor_scalar_mul(
                    out=gam[:], in0=rec[:], scalar1=0.5
                )
                bet = p3c.tile([P, nt3], f32, name="bet")
                nc.vector.tensor_tensor(
                    out=bet[:], in0=cntc[:], in1=rec[:], op=mult
                )
                nc.vector.tensor_scalar(
                    out=bet[:], in0=bet[:],
                    scalar1=-0.5, scalar2=1.0, op0=mult, op1=add,
                )

                for tt in range(nt3):
                    w = min(P, cs - tt * P)
                    trp = psum3.tile([P, d], f32, name=f"trp_{tt}", tag="trp",
                                     space="PSUM")
                    nc.tensor.transpose(
                        out=trp[0:w, 0:d],
                        in_=st_sb[:, tt * P:tt * P + w],
                        identity=identity[:, 0:d],
                    )
                    o1 = p3.tile([P, d], f32, name=f"o1_{tt}", tag="o1")
                    nc.scalar.activation(
                        o1[0:w, :],
                        ctr_sb[0:w, tt * d:tt * d + d],
                        mybir.ActivationFunctionType.Copy,
                        scale=bet[0:w, tt:tt + 1],
                    )
                    ou = p3.tile([P, d], f32, name=f"ou_{tt}", tag="ou")
                    nc.vector.scalar_tensor_tensor(
                        out=ou[0:w, :], in0=trp[0:w, 0:d],
                        scalar=gam[0:w, tt:tt + 1],
                        in1=o1[0:w, :], op0=mult, op1=add,
                    )
                    nc.sync.dma_start(
                        out=out[tt * P:tt * P + w, :], in_=ou[0:w, 0:d]
                    )

    nc.compile()
    return nc


_PROGRAM = None
LAST_RESULTS = None  # BassKernelResults from the most recent run (for test.py)


def _get_program():
    global _PROGRAM
    if _PROGRAM is None:
        _PROGRAM = build_program()
    return _PROGRAM


def kernel(embeded_preds, labels, center):
    from concourse.bass_utils import run_bass_kernel_spmd

    global LAST_RESULTS
    preds = np.asarray(embeded_preds, dtype=np.float32)
    lab = np.asarray(labels, dtype=np.float32)
    ctr = np.ascontiguousarray(np.asarray(center, dtype=np.float32))
    assert preds.shape == (N, D) and lab.shape == (N, C) and ctr.shape == (C, D)

    # pack preds: partition p, col s*(R*D)+q*D+d  <-  preds[P*R*s + R*p + q, d]
    preds_packed = np.ascontiguousarray(
        preds.reshape(ST, P, R * D).transpose(1, 0, 2).reshape(P, ST * R * D)
    )

    nc = _get_program()
    in_maps = [
        {
            "preds": preds_packed,
            "labels": np.ascontiguousarray(
                lab[:, i * CS:(i + 1) * CS]
            ).reshape(ST * P, R * CS),
            "center": ctr[i * CS:(i + 1) * CS],
        }
        for i in range(NCORES)
    ]
    res = run_bass_kernel_spmd(nc, in_maps, core_ids=list(range(NCORES)))
    LAST_RESULTS = res
    return np.concatenate([res.results[i]["out"] for i in range(NCORES)], axis=0)


# revision 18
# speedup vs baseline: 1.1831x; 1.1831x over previous
"""CenterLoss update kernel for Trainium2, 8-core SPMD — class-sharded, collective-free.

Reference computation (N=16384 samples, C=10000 classes, D=128 dims):
    embeded_labels = labels @ center          # [N,D] gather via one-hot
    diff = embeded_labels - embeded_preds
    grad = (labels.T @ diff) / (counts + 1)   # counts = labels.T @ ones
    out  = center - 0.5 * grad

Because each row of ``labels`` is one-hot, ``labels.T @ labels == diag(counts)``,
so the whole thing collapses to a single pass over ``labels``:

    S      = labels.T @ embeded_preds         # [C,D] per-class sum of preds
    counts = column sums of labels            # [C]
    out    = beta * center + gamma * S
             beta  = 1 - 0.5*counts/(counts+1)
             gamma = 0.5/(counts+1)

Sharding: each core owns C/8 = 1250 classes and reads ALL N rows of its
1250-column slice of ``labels`` (same total HBM traffic as batch-sharding)
plus a replicated copy of ``preds`` (8.4MB).  No inter-core communication at
all — the batch-sharded variant needs a [C,D] ReduceScatter that shows up as
a ~100us serial tail with HBM nearly idle.

The kernel streams 91.6MB/core from HBM at ~356 GB/s (near the per-core
share of HBM bandwidth), so everything else must hide under the stream:

* The host packs each core's label slice as [4096, 5000] (4 consecutive
  sample-rows per SBUF partition line) so one super-tile DMA moves 2.56MB
  with 20KB contiguous per-partition segments.  ``preds`` is host-packed to
  match: partition p, column s*512+q*128+d holds preds[512s+4p+q, d], making
  the matmul stationary operand for sub-step (s, q) a plain SBUF slice.
* Labels stream through the PE once as the moving operand of a single fp32r
  pass (1 row/cycle; the 2e-2 rel-err budget makes the fp32 hi/lo
  double-pass unnecessary), accumulating S.T = preds.T @ labels in PSUM.
  The stationary preds must also be f32r: the BIR verifier rejects mixing
  32-bit and non-32-bit matmul inputs, so the cheap bf16 weight-load path
  is unavailable.
* Counts accumulate on the DVE in bf16 (exact for one-hot partial counts),
  pre-folding each super-tile's 4 row-blocks to 2500 columns so the final
  partition-reduce matmul has a short bf16 moving operand.
* The epilogue transposes per-class counts into a [128, 10] block so
  beta/gamma come from 4 batched DVE ops instead of 40 short ones, then
  does one ACT op + one DVE op + one store per 128-class tile.
"""

import numpy as np

N, C, D = 16384, 10000, 128
NCORES = 8
CS = C // NCORES   # 1250 classes per core
LR = 0.5
P = 128
R = 4              # sample-rows packed per SBUF partition line
ST = N // (P * R)  # 32 label super-tiles of [128, R*CS]


def _chunks(width, step=512):
    out = []
    c0 = 0
    while c0 < width:
        out.append((c0, min(step, width - c0)))
        c0 += step
    return out


# matmul chunks over the 1250-class shard, padded so every chunk is >=256
# wide: fp32r moving operands narrower than 256 run at 2-4 cycles/row.  The
# pad columns multiply garbage SBUF into PSUM columns [1250, 1280) that are
# never read.
MM_CHUNKS = [(0, 512), (512, 512), (1024, 256)]
CSP = 1280  # padded class width (PSUM tile / lab tile per-block stride)


def build_program(n=N, cs=CS, d=D, ncores=NCORES):
    """Build the SPMD Bass program (identical on every core)."""
    import concourse.bacc as bacc
    import concourse.mybir as mybir
    import concourse.tile as tile
    from concourse.masks import make_identity

    f32 = mybir.dt.float32
    f32r = mybir.dt.float32r
    bf16 = mybir.dt.bfloat16
    mult = mybir.AluOpType.mult
    add = mybir.AluOpType.add

    st = n // (P * R)          # label super-tiles
    gw = R * cs                # super-tile width (5000)
    hw_ = gw // 2              # folded counts width (2500)
    nt3 = (cs + P - 1) // P    # epilogue tiles over the class shard (10)
    npr = 4                    # preds split into 4 DMA tiles
    prw = (st * R * d) // npr  # 4096 cols per preds tile

    nc = bacc.Bacc(
        "TRN2",
        target_bir_lowering=False,
        debug=False,
        num_devices=ncores,
    )

    # labels are consumed by fp32r matmuls; declaring the DRAM tensor f32r
    # (same 32-bit values, trivially rounded) lets plain HWDGE DMAs feed the
    # PE at full speed -- the SWDGE cast path is descriptor-bound.
    # Host-side layouts (see kernel()):
    #   labels [4096, 5000]: row r = label rows 4r..4r+3 of this core's
    #                        1250-class column slice, concatenated
    #   preds  [128, 16384]: partition p, col s*512+q*128+d = preds[512s+4p+q, d]
    preds = nc.dram_tensor("preds", [P, st * R * d], f32r, kind="ExternalInput").ap()
    labels = nc.dram_tensor("labels", [st * P, gw], f32r, kind="ExternalInput").ap()
    center = nc.dram_tensor("center", [cs, d], f32, kind="ExternalInput").ap()
    out = nc.dram_tensor("out", [cs, d], f32, kind="ExternalOutput").ap()

    with tile.TileContext(nc) as tc:
        with tc.tile_pool(name="const", bufs=1) as const_pool:
            identity = const_pool.tile([P, P], f32, name="identity")
            make_identity(nc, identity[:])
            ones_col = const_pool.tile([P, 1], bf16, name="ones_col")
            nc.vector.memset(ones_col[:], 1.0)

            # preds, host-packed; 4 DMAs of ~2.1MB on the ACT HWDGE ring so
            # the label stream on the SP ring is never queued behind them.
            preds_sb = []
            for t in range(npr):
                pt = const_pool.tile([P, prw], f32r, name=f"preds_{t}")
                nc.scalar.dma_start(
                    out=pt[:], in_=preds[:, t * prw:(t + 1) * prw]
                )
                preds_sb.append(pt)

            # center shard, as nt3 [class, d] tiles (class on partitions)
            ctr_sb = const_pool.tile([P, nt3 * d], f32, name="ctr_sb")
            for tt in range(nt3):
                w = min(P, cs - tt * P)
                nc.scalar.dma_start(
                    out=ctr_sb[0:w, tt * d:tt * d + d],
                    in_=center[tt * P:tt * P + w, :],
                )

            # folded per-partition partial counts (bf16 is exact: integers
            # well under 256); col h*1250+j accumulates classes j from two of
            # the four row-blocks
            counts_sb = const_pool.tile([P, hw_], bf16, name="counts_sb")
            st_sb = const_pool.tile([d, cs], f32, name="st_sb")
            cnt_row = const_pool.tile([1, cs], f32, name="cnt_row")

            # ---------------- phase 1: stream labels ----------------
            with (
                tc.tile_pool(name="lab", bufs=4) as lab_pool,
                tc.tile_pool(name="fold", bufs=2) as fold_pool,
                tc.tile_pool(name="psum1", bufs=1, space="PSUM") as psum1,
            ):
                st_psum = psum1.tile([d, CSP], f32, name="st_psum", space="PSUM")
                for s in range(st):
                    # tile is gw+pad wide; the DMA fills [0, gw) and the
                    # padded matmul chunks read up to 30 garbage columns
                    lab_t = lab_pool.tile(
                        [P, gw + (CSP - cs)], f32r, name=f"lab_{s}", tag="lab"
                    )
                    nc.sync.dma_start(
                        out=lab_t[:, 0:gw], in_=labels[s * P:(s + 1) * P, :]
                    )
                    for q in range(R):
                        col = s * (R * d) + q * d
                        pt = preds_sb[col // prw]
                        off = col % prw
                        for c0, w in MM_CHUNKS:
                            nc.tensor.matmul(
                                out=st_psum[:, c0:c0 + w],
                                lhsT=pt[:, off:off + d],
                                rhs=lab_t[:, q * cs + c0:q * cs + c0 + w],
                                start=(s == 0 and q == 0),
                                stop=(s == st - 1 and q == R - 1),
                            )
                    # counts: fold row-blocks {0,1}+{2,3} (fp32 reads, bf16
                    # write), then accumulate bf16+bf16 at the DVE 2x rate
                    if s == 0:
                        nc.vector.tensor_tensor(
                            out=counts_sb[:],
                            in0=lab_t[:, 0:hw_].bitcast(f32),
                            in1=lab_t[:, hw_:gw].bitcast(f32),
                            op=add,
                        )
                    else:
                        tmp = fold_pool.tile([P, hw_], bf16, name=f"tmp_{s}",
                                             tag="tmp")
                        nc.vector.tensor_tensor(
                            out=tmp[:],
                            in0=lab_t[:, 0:hw_].bitcast(f32),
                            in1=lab_t[:, hw_:gw].bitcast(f32),
                            op=add,
                        )
                        nc.vector.tensor_add(
                            out=counts_sb[:], in0=counts_sb[:], in1=tmp[:]
                        )

                # counts: reduce 128 partitions with a ones matmul,
                # PSUM-accumulating the two folded halves
                cnt_psum = psum1.tile([1, cs], f32, name="cnt_psum", space="PSUM")
                for h in range(2):
                    for c0, w in _chunks(cs):
                        nc.tensor.matmul(
                            out=cnt_psum[0:1, c0:c0 + w],
                            lhsT=ones_col[:],
                            rhs=counts_sb[:, h * cs + c0:h * cs + c0 + w],
                            start=(h == 0),
                            stop=(h == 1),
                        )
                nc.scalar.copy(out=st_sb[:], in_=st_psum[:, 0:cs])
                nc.scalar.copy(out=cnt_row[:], in_=cnt_psum[:])

            # ---------------- phase 2: per-class update ----------------
            with (
                tc.tile_pool(name="p3", bufs=2) as p3,
                tc.tile_pool(name="p3c", bufs=1) as p3c,
                tc.tile_pool(name="psum3", bufs=2, space="PSUM") as psum3,
                tc.tile_pool(name="psumc", bufs=1, space="PSUM") as psumc,
            ):
                # gather per-class counts into [128, nt3] so beta/gamma are
                # 4 batched DVE ops instead of 4*nt3 short ones
                cntc = psumc.tile([P, nt3], f32, name="cntc", space="PSUM")
                for tt in range(nt3):
                    w = min(P, cs - tt * P)
                    nc.tensor.transpose(
                        out=cntc[0:w, tt:tt + 1],
                        in_=cnt_row[0:1, tt * P:tt * P + w],
                        identity=identity[0:1, 0:1],
                    )
                rec = p3c.tile([P, nt3], f32, name="rec")
                nc.vector.tensor_scalar_add(
                    out=rec[:], in0=cntc[:], scalar1=1.0
                )
                nc.vector.reciprocal(out=rec[:], in_=rec[:])
                gam = p3c.tile([P, nt3], f32, name="gam")
                nc.vector.tensor_scalar_mul(
                    out=gam[:], in0=rec[:], scalar1=0.5
                )
                bet = p3c.tile([P, nt3], f32, name="bet")
                nc.vector.tensor_tensor(
                    out=bet[:], in0=cntc[:], in1=rec[:], op=mult
                )
                nc.vector.tensor_scalar(
                    out=bet[:], in0=bet[:],
                    scalar1=-0.5, scalar2=1.0, op0=mult, op1=add,
                )

                for tt in range(nt3):
                    w = min(P, cs - tt * P)
                    trp = psum3.tile([P, d], f32, name=f"trp_{tt}", tag="trp",
                                     space="PSUM")
                    nc.tensor.transpose(
                        out=trp[0:w, 0:d],
                        in_=st_sb[:, tt * P:tt * P + w],
                        identity=identity[:, 0:d],
                    )
                    o1 = p3.tile([P, d], f32, name=f"o1_{tt}", tag="o1")
                    nc.scalar.activation(
                        o1[0:w, :],
                        ctr_sb[0:w, tt * d:tt * d + d],
                        mybir.ActivationFunctionType.Copy,
                        scale=bet[0:w, tt:tt + 1],
                    )
                    ou = p3.tile([P, d], f32, name=f"ou_{tt}", tag="ou")
                    nc.vector.scalar_tensor_tensor(
                        out=ou[0:w, :], in0=trp[0:w, 0:d],
                        scalar=gam[0:w, tt:tt + 1],
                        in1=o1[0:w, :], op0=mult, op1=add,
                    )
                    nc.sync.dma_start(
                        out=out[tt * P:tt * P + w, :], in_=ou[0:w, 0:d]
                    )

    nc.compile()
    return nc


_PROGRAM = None
LAST_RESULTS = None  # BassKernelResults from the most recent run (for test.py)


def _get_program():
    global _PROGRAM
    if _PROGRAM is None:
        _PROGRAM = build_program()
    return _PROGRAM


def kernel(embeded_preds, labels, center):
    from concourse.bass_utils import run_bass_kernel_spmd

    global LAST_RESULTS
    preds = np.asarray(embeded_preds, dtype=np.float32)
    lab = np.asarray(labels, dtype=np.float32)
    ctr = np.ascontiguousarray(np.asarray(center, dtype=np.float32))
    assert preds.shape == (N, D) and lab.shape == (N, C) and ctr.shape == (C, D)

    # pack preds: partition p, col s*(R*D)+q*D+d  <-  preds[P*R*s + R*p + q, d]
    preds_packed = np.ascontiguousarray(
        preds.reshape(ST, P, R * D).transpose(1, 0, 2).reshape(P, ST * R * D)
    )

    nc = _get_program()
    in_maps = [
        {
            "preds": preds_packed,
            "labels": np.ascontiguousarray(
                lab[:, i * CS:(i + 1) * CS]
            ).reshape(ST * P, R * CS),
            "center": ctr[i * CS:(i + 1) * CS],
        }
        for i in range(NCORES)
    ]
    res = run_bass_kernel_spmd(nc, in_maps, core_ids=list(range(NCORES)))
    LAST_RESULTS = res
    return np.concatenate([res.results[i]["out"] for i in range(NCORES)], axis=0)
